# revision 1
# baseline (speedup 1.0000x reference)
"""MultiHeadAttention (B=2, S=2048, D=1024, H=16) on 8 trn2 cores.

Tensor-parallel over heads: core c owns heads 2c, 2c+1 (128 output features).
Per core:
  phase A: project q/k/v transposed:  qT = (Wq_c/8) @ X^T   [128 feat, 4096 tok]
           (X^T streamed from DRAM in bf16; W shards resident)
           v additionally PE-transposed to natural layout and augmented with a
           ones column per 128-token chunk (rowsum rides the attn@v matmul).
  phase B: per (batch, head):  S^T = kT^T-chunks @ qT  (scores transposed:
           key-tokens on partitions, query-tokens free)  ->  exp on ScalarE
           -> multiply by notmask (bf16, DVE) -> out^T[65, n] += v_aug^T @ expT
           accumulated over key chunks in PSUM.  out^T row 64 is the softmax
           denominator.  Division happens on host in fp32.
"""

import sys

sys.path.insert(0, "/opt/trn_rl_repo")

import numpy as np
import ml_dtypes

import concourse.mybir as mybir
import concourse.tile as tile
from concourse import bacc
from concourse.bass_utils import run_bass_kernel_spmd
from concourse.masks import make_identity

BF16 = mybir.dt.bfloat16
F32 = mybir.dt.float32
NP_BF16 = ml_dtypes.bfloat16

NCORES = 8
B, S, D = 2, 2048, 1024
H, DH = 16, 64
HPC = H // NCORES  # heads per core = 2
MPC = HPC * DH  # output features per core = 128
T = B * S  # 4096 tokens
NKC = D // 128  # 8 contraction chunks for projections
NNC = T // 512  # 8 token chunks of 512 (projection N tiling)
NJC = S // 128  # 16 key-token chunks per batch
NTC = T // 128  # 32 global token chunks (v_aug)
VW = DH + 1  # 65: head dim + ones column

_CACHE: dict = {}


def _emit(nc, dins, dout):
    from contextlib import ExitStack

    tc = dins["_tc"]
    NCB = S // 512  # 4 token chunks of 512 per batch
    with ExitStack() as ctx:
        singles = ctx.enter_context(tc.tile_pool(name="singles", bufs=1))

        w_sb, b_sb = {}, {}
        for t in ("q", "k", "v"):
            w = singles.tile([128, NKC, 128], BF16, tag=f"w{t}")
            nc.sync.dma_start(
                out=w,
                in_=dins[f"w{t}T"].ap().rearrange("(c p) m -> p c m", p=128),
            )
            w_sb[t] = w
            bb = singles.tile([128, 1], F32, tag=f"b{t}")
            nc.sync.dma_start(
                out=bb, in_=dins[f"b{t}"].ap().rearrange("(p o) -> p o", o=1)
            )
            b_sb[t] = bb

        qT = singles.tile([128, T], BF16, tag="qT")
        vT = singles.tile([128, T], BF16, tag="vT")
        # Packed kT: head h's 64 rows live at partitions h*64..h*64+64 (no
        # zero padding -- K=64 matmuls with base_partition h*64 run at the
        # same 1 col/cycle as K=128, verified on hw).  Saves two 4096-col
        # DVE memsets on the startup critical path and one [128,512] Act
        # drain per proj chunk.
        kTp = singles.tile([128, T], BF16, tag="kTp")

        v_aug = [
            singles.tile([128, NTC * VW], BF16, tag=f"vaug{h}", name=f"vaug{h}")
            for h in range(HPC)
        ]
        for h in range(HPC):
            nc.vector.memset(v_aug[h], 1.0)

        ident = singles.tile([128, 128], BF16, tag="ident")
        make_identity(nc, ident)

        # batch-resident transposed notmask, one tile per batch
        nm_sb = [
            singles.tile([128, NJC, S], BF16, tag=f"nm{b}", name=f"nm{b}")
            for b in range(B)
        ]

        xp = ctx.enter_context(tc.tile_pool(name="xpanels", bufs=3))
        expp = ctx.enter_context(tc.tile_pool(name="expp", bufs=4))
        outsb = ctx.enter_context(tc.tile_pool(name="outsb", bufs=1))
        # PSUM budget (8 banks total): proj/vt shared tag 2, scores 4, out 2
        psA = ctx.enter_context(tc.tile_pool(name="psA", bufs=2, space="PSUM"))
        psS = ctx.enter_context(tc.tile_pool(name="psS", bufs=2, space="PSUM"))
        psO = ctx.enter_context(tc.tile_pool(name="psO", bufs=1, space="PSUM"))

        # PE p-state warm-up: dummy matmuls on the (tiny, early-arriving)
        # weight tiles keep the tensor engine busy while the first x panel
        # streams in, so real matmuls start at a ramped clock instead of
        # the cold 0.65 GHz p-state.  (Not ident: make_identity waits on
        # the GpSimd instruction-table load, ~3us into the kernel.)
        warm = psA.tile([128, 512], F32, tag="proj", name="warm")
        for _ in range(10):
            nc.tensor.matmul(
                warm[:, :128],
                lhsT=w_sb["q"][:, 0, :],
                rhs=w_sb["k"][:, 0, :],
                start=True,
                stop=True,
            )

        def proj_chunk(t, b, ncb, drain_dve=False):
            """Project one 512-token chunk of tensor t: X^T panel -> projT."""
            col = b * S + ncb * 512
            xtile = xp.tile([128, NKC, 512], BF16, tag="xpanel", name="xpanel")
            nc.sync.dma_start(
                out=xtile,
                in_=dins[f"x{t}T"]
                .ap()[:, col : col + 512]
                .rearrange("(c p) n -> p c n", p=128),
            )
            ps = psA.tile([128, 512], F32, tag="proj")
            for kc in range(NKC):
                nc.tensor.matmul(
                    ps,
                    lhsT=w_sb[t][:, kc, :],
                    rhs=xtile[:, kc, :],
                    start=(kc == 0),
                    stop=(kc == NKC - 1),
                )
            dst = {"q": qT, "k": kTp, "v": vT}[t]
            if drain_dve:
                # interleaved chunks drain on DVE (bias via stride-0
                # broadcast) so the exp-paced Act stream isn't perturbed
                nc.vector.tensor_add(
                    dst[:, col : col + 512],
                    ps,
                    b_sb[t].broadcast_to([128, 512]),
                )
            else:
                # drain on ScalarE (idle during serial projection phases)
                nc.scalar.activation(
                    out=dst[:, col : col + 512],
                    in_=ps,
                    func=mybir.ActivationFunctionType.Identity,
                    bias=b_sb[t],
                )

        def v_transpose(b, ncb):
            """PE-transpose 512 projected v columns into v_aug (4 chunks)."""
            tbase = b * NJC + ncb * 4
            pst = psA.tile([128, 512], BF16, tag="proj", name="vtps")
            for i in range(4):
                nc.tensor.transpose(
                    out=pst[:, i * 128 : (i + 1) * 128],
                    in_=vT[:, (tbase + i) * 128 : (tbase + i + 1) * 128],
                    identity=ident,
                )
            for h in range(HPC):
                # strided copy: 4 chunks x 64 head cols -> v_aug stride-65 slots
                src = pst.rearrange("p (i d) -> p i d", i=4)[:, :, h * DH : (h + 1) * DH]
                dst = v_aug[h][:, tbase * VW : (tbase + 4) * VW].rearrange(
                    "p (i w) -> p i w", i=4
                )[:, :, 0:DH]
                nc.vector.tensor_copy(out=dst, in_=src)

        def emit_proj(b):
            for ncb in range(NCB):
                proj_chunk("k", b, ncb)
                proj_chunk("v", b, ncb)
                v_transpose(b, ncb)
                proj_chunk("q", b, ncb)

        def proj_unit(t, b, ncb):
            if t == "vt":
                return lambda: v_transpose(b, ncb)
            return lambda: proj_chunk(t, b, ncb, True)

        def emit_mask_dma(b):
            for jc in range(NJC):
                nc.sync.dma_start(
                    out=nm_sb[b][:, jc, :],
                    in_=dins["nmT"].ap()[b, jc * 128 : (jc + 1) * 128, :],
                )

        def emit_drain(outps, b, h, nh):
            # drain in 512-halves so each half's store DMA starts as
            # soon as that half is copied (shorter kernel tail).
            # The store DMA is issued by the idle GpSimd engine: a
            # sync-queue dma_start would sit in the sync stream
            # WAITING on the copy sem, head-of-line-blocking every
            # later descriptor (interleaved x panels, mask chunks).
            osb = outsb.tile([VW, 1024], F32, tag="osb")
            for s2 in range(2):
                nc.vector.tensor_copy(
                    out=osb[:, s2 * 512 : (s2 + 1) * 512],
                    in_=outps[:, s2 * 512 : (s2 + 1) * 512],
                )
                nc.gpsimd.dma_start(
                    out=dout.ap()[
                        b, h, :, nh * 1024 + s2 * 512 : nh * 1024 + (s2 + 1) * 512
                    ],
                    in_=osb[:, s2 * 512 : (s2 + 1) * 512],
                )

        def emit_attn(b, units=(), slots=()):
            """Attention for batch b; `units` (the other batch's projection
            chunks) are interleaved at jc-tile indices `slots` so their PE
            work hides under the exp-paced attention pipeline instead of
            occupying its own serial phase.  A group's psO drain is emitted
            lazily, after the NEXT group's first exp/mask, so the two DVE
            drain copies don't delay that group's first et tile (the psO
            tile itself is allocated after the drain so the pool's WAR
            tracking stays correct)."""
            units = list(units)
            slots = list(slots)
            ui = 0
            tile_i = -1
            pending = None
            for nh in range(2):
                nbase = b * S + nh * 1024
                for h in range(HPC):
                    outps = None
                    for jc in range(NJC):
                        tile_i += 1
                        while ui < len(units) and slots[ui] <= tile_i:
                            units[ui]()
                            ui += 1
                        tglob = b * NJC + jc
                        ps = psS.tile([128, 1024], F32, tag="scores")
                        for s2 in range(2):
                            nc.tensor.matmul(
                                ps[:, s2 * 512 : (s2 + 1) * 512],
                                lhsT=kTp[
                                    h * DH : (h + 1) * DH,
                                    tglob * 128 : (tglob + 1) * 128,
                                ],
                                rhs=qT[
                                    h * DH : (h + 1) * DH,
                                    nbase + s2 * 512 : nbase + (s2 + 1) * 512,
                                ],
                                start=True,
                                stop=True,
                            )
                        et = expp.tile([128, 1024], BF16, tag="exp")
                        nc.scalar.activation(
                            out=et, in_=ps, func=mybir.ActivationFunctionType.Exp
                        )
                        nc.vector.tensor_mul(
                            et, et, nm_sb[b][:, jc, nh * 1024 : (nh + 1) * 1024]
                        )
                        if jc == 0:
                            if pending is not None:
                                emit_drain(*pending)
                                pending = None
                            outps = psO.tile(
                                [VW, 1024], F32, tag="out", name="outps"
                            )
                        for s2 in range(2):
                            nc.tensor.matmul(
                                outps[:, s2 * 512 : (s2 + 1) * 512],
                                lhsT=v_aug[h][:, tglob * VW : tglob * VW + VW],
                                rhs=et[:, s2 * 512 : (s2 + 1) * 512],
                                start=(jc == 0),
                                stop=(jc == NJC - 1),
                            )
                    pending = (outps, b, h, nh)
            while ui < len(units):
                units[ui]()
                ui += 1
            emit_drain(*pending)

        def mask_chunk_unit(b, jc):
            return lambda: nc.sync.dma_start(
                out=nm_sb[b][:, jc, :],
                in_=dins["nmT"].ap()[b, jc * 128 : (jc + 1) * 128, :],
            )

        for _ in range(dins.get("_repeat", 1)):
            # proj0 first half: no mask competition (panel prefetch is
            # filling); second half: two mask0 chunk descriptors ride after
            # each unit, so the 8MB mask0 stream starts ~10us earlier than
            # a block emission at proj0's end (the sync stream trickles
            # through panel starts at PE pace) and attn0's first group
            # never races a cold mask stream.
            for ncb in (0, 1):
                proj_chunk("k", 0, ncb)
                proj_chunk("v", 0, ncb)
                v_transpose(0, ncb)
                proj_chunk("q", 0, ncb)
            mi = 0
            for ncb in (2, 3):
                for t in ("k", "v", "vt", "q"):
                    if t == "vt":
                        v_transpose(0, ncb)
                    else:
                        proj_chunk(t, 0, ncb)
                    for _m in range(2):
                        mask_chunk_unit(0, mi)()
                        mi += 1
            # Batch-1 projection AND batch-1 mask chunks ride the attn0
            # window (one of each per 4 jc-tiles): the proj PE work hides
            # under the exp-paced attention pipeline instead of occupying
            # its own serial phase, and the 8MB mask stream is paced
            # smoothly across the window (it would otherwise either start
            # at attn0's end, starving attn1's first group, or dump all at
            # once, starving the latency-critical panel/mask0 flows).
            u = []
            for i, (t, n) in enumerate(
                (t, n) for n in range(NCB) for t in ("k", "v", "vt", "q")
            ):
                u.append((proj_unit(t, 1, n), 4 * (i + 1)))
                u.append((mask_chunk_unit(1, i), 4 * (i + 1)))
            emit_attn(0, units=[x[0] for x in u], slots=[x[1] for x in u])
            emit_attn(1)


def _build(repeat=1):
    key = ("nc", repeat)
    if key in _CACHE:
        return _CACHE[key]
    nc = bacc.Bacc("TRN2", target_bir_lowering=False, debug=False)
    dins = {}
    for t in ("q", "k", "v"):
        dins[f"x{t}T"] = nc.dram_tensor(f"x{t}T", [D, T], BF16, kind="ExternalInput")
        dins[f"w{t}T"] = nc.dram_tensor(f"w{t}T", [D, MPC], BF16, kind="ExternalInput")
        dins[f"b{t}"] = nc.dram_tensor(f"b{t}", [MPC], F32, kind="ExternalInput")
    dins["nmT"] = nc.dram_tensor("nmT", [B, S, S], BF16, kind="ExternalInput")
    dout = nc.dram_tensor("out", [B, HPC, VW, S], F32, kind="ExternalOutput")

    with tile.TileContext(nc) as tc:
        dins["_tc"] = tc
        dins["_repeat"] = repeat
        _emit(nc, dins, dout)
        del dins["_tc"], dins["_repeat"]
    nc.compile()
    _CACHE[key] = nc
    return nc


def _prep_inputs(query, key, value, mask, Wq, bq, Wk, bk, Wv, bv):
    """Host-side shard prep. Returns per-core input maps."""
    xs = {}
    for name, x in (("q", query), ("k", key), ("v", value)):
        xt = np.ascontiguousarray(
            np.asarray(x, dtype=np.float32).reshape(T, D).T
        ).astype(NP_BF16)
        xs[f"x{name}T"] = xt

    nm = (~np.asarray(mask)).astype(NP_BF16)
    nmT = np.ascontiguousarray(np.transpose(nm, (0, 2, 1)))

    Wq = np.asarray(Wq, dtype=np.float32)
    Wk = np.asarray(Wk, dtype=np.float32)
    Wv = np.asarray(Wv, dtype=np.float32)
    bq = np.asarray(bq, dtype=np.float32)
    bk = np.asarray(bk, dtype=np.float32)
    bv = np.asarray(bv, dtype=np.float32)
    scale = 1.0 / np.sqrt(np.float32(DH))

    in_maps = []
    for c in range(NCORES):
        r = slice(c * MPC, (c + 1) * MPC)
        m = dict(xs)
        m["nmT"] = nmT
        m["wqT"] = np.ascontiguousarray((Wq[r] * scale).T).astype(NP_BF16)
        m["wkT"] = np.ascontiguousarray(Wk[r].T).astype(NP_BF16)
        m["wvT"] = np.ascontiguousarray(Wv[r].T).astype(NP_BF16)
        m["bq"] = np.ascontiguousarray(bq[r] * scale)
        m["bk"] = np.ascontiguousarray(bk[r])
        m["bv"] = np.ascontiguousarray(bv[r])
        in_maps.append(m)
    return in_maps


def _assemble(results):
    """results: per-core dicts with 'out' [B, HPC, 65, S] f32 -> [B, S, D]."""
    full = np.empty((B, S, D), dtype=np.float32)
    for c in range(NCORES):
        o = results[c]["out"]
        for b in range(B):
            for h in range(HPC):
                num = o[b, h, :DH, :]  # [64, S]
                den = o[b, h, DH, :]  # [S]
                col = c * MPC + h * DH
                full[b, :, col : col + DH] = (num / den).T
    return full


def kernel(query, key, value, mask, Wq, bq, Wk, bk, Wv, bv, **extra):
    nc = _build()
    in_maps = _prep_inputs(query, key, value, mask, Wq, bq, Wk, bk, Wv, bv)
    res = run_bass_kernel_spmd(nc, in_maps, core_ids=list(range(NCORES)))
    return _assemble(res.results)


def run_traced(inputs, **trace_kwargs):
    """For test.py: run with NTFF tracing, return (output, BassKernelResults)."""
    nc = _build()
    in_maps = _prep_inputs(**{k: inputs[k] for k in (
        "query", "key", "value", "mask", "Wq", "bq", "Wk", "bk", "Wv", "bv")})
    try:
        res = run_bass_kernel_spmd(
            nc, in_maps, core_ids=list(range(NCORES)), trace=True, **trace_kwargs
        )
    except ModuleNotFoundError:
        res = run_bass_kernel_spmd(nc, in_maps, core_ids=list(range(NCORES)))
    return _assemble(res.results), res

